# revision 6
# baseline (speedup 1.0000x reference)
"""Trainium2 Bass kernel for y = 2*(einsum('bct,oc->bot', pre, W_pre) + b_pre).

Shapes (hardcoded): pre [16, 512, 4096] f32, W_pre [512, 512] f32, b_pre [512] f32.
Sharding: data-parallel over B across 8 cores (2 batches per core).

Per core: out[b, o, t] = 2*(sum_c W[o,c]*pre[b,c,t] + bias[o]) for 2 batches.
PE matmul computes lhsT.T @ rhs with lhsT = W.T tiles [K=128, M=128] and
rhs = pre column windows [K=128, N<=512]; 4 K-tiles accumulate into one
PSUM bank, then ScalarE/DVE apply out = psum + 2*bias on eviction
PSUM->SBUF (the reference's y+y is folded into W on the host).

I/O runs in fp16: per core f32 I/O moves 33.6 MB (~94 us at the 358 GB/s
HBM limit) while the PE needs only ~55 us at 1 cycle/row, so f32 is
DMA-bound; fp16 halves traffic and runs at the same PE rate.

Schedule notes (from NTFF traces):
- HWDGE dma_start costs ~650 ns fixed issue + descriptor-count-driven
  doorbell latency, so the host pre-blocks `pre` per column chunk
  ([p, kt, j] flattened) making every x chunk one contiguous-line
  128-descriptor DMA, and pre-tiles W per output column so the first
  weight tile (128 KB) lands as early as possible.
- x loads issue on sync, weights/bias on scalar (the two HWDGE queues),
  bulk stores on gpsimd (SWDGE). Concurrent transfers share the 16 SDMA
  engines, so the first x windows are tiny (128 cols) to land early.
- The PE HAM clock gate starts at 1.2 GHz and reaches 2.4 GHz only after
  ~3.4 us of sustained matmul activity. A burst of dummy matmuls on
  scratch SBUF issued before the main loop warms the PE during the
  ~2.8 us DMA head, so real matmuls run at full rate almost from the
  start; the burst also bridges to first-data so the HAM busy window
  isn't reset by an idle gap.
- Tail: the final 512-col group evicts in 256-col halves on DVE+ACT in
  parallel and stores per M-tile (4 small DMAs) on the by-then-idle sync
  queue, so the last HBM write chases the last matmul as closely as
  possible before the fixed ~2.5 us teardown barrier.
"""

import os
import sys

for _p in ("/opt/trn_rl_repo", "/root/.axon_site/_ro/trn_rl_repo"):
    if os.path.isdir(_p) and _p not in sys.path:
        sys.path.append(_p)

from contextlib import ExitStack

import numpy as np

import concourse.bass as bass
import concourse.tile as tile
from concourse import bacc, mybir
from concourse.bass_utils import run_bass_kernel_spmd

B, C, T = 16, 512, 4096  # batch, channels (in == out), sequence
NCORES = 8
BPC = B // NCORES  # batches per core
P = 128
KT = C // P  # contraction tiles
MT = C // P  # output-channel tiles
NCHUNK = 512  # max matmul moving-operand free dim (one PSUM bank of f32)
# Input DMA column chunks: tiny first chunks so the first matmul group's
# data lands early despite SDMA sharing, bigger later ones to amortize the
# fixed HWDGE issue cost.
XCS = [128, 128, 256, 512, 1024, 2048]
NDUMMY = 12  # warmup matmuls (N=256): ~12 * 236 ns cold bridges to first data

IO_DT = mybir.dt.float16  # matmul + DRAM I/O dtype

LAST_RESULT = None  # BassKernelResults of the most recent run (for test harness)
_cache = {}


def _windows():
    """Column windows (xi, xoff, ncols) covering [0, T) in matmul-N pieces."""
    ws = []
    for xi, xcols in enumerate(XCS):
        for o in range(0, xcols, NCHUNK):
            ws.append((xi, o, min(NCHUNK, xcols - o)))
    assert sum(w[2] for w in ws) == T
    return ws


# Output store groups as window-index ranges per batch. Batch 1 tapers so
# the final DMAs after the last matmul are small.
WINDOWS = _windows()  # 10 windows: 128,128,256,512 x7
OG_SPLIT = {0: [6, 4], 1: [6, 2, 1, 1]}  # cols 2048/2048 and 2048/1024/512/512


def _build():
    # Bacc (not plain Bass): its finalize() runs move_matmul_waits_to_ldweights +
    # generate_event_semaphores, which walrus needs.
    nc = bacc.Bacc("TRN2", target_bir_lowering=False, debug=False, num_devices=NCORES)
    # Host layout: prex[b, p, 4*off + kt*xcols + j] for chunk at column off —
    # each chunk is a contiguous [128, 4*xcols] block (128 descriptors).
    prex = nc.dram_tensor("prex", [BPC, P, KT * T], IO_DT, kind="ExternalInput").ap()
    # Host layout: wt[mt, p, kt*128 + m] = 2*W.T[kt*128+p, mt*128+m] — one
    # contiguous [128, 512] load per output-column tile.
    wt = nc.dram_tensor("wt", [MT, P, KT * P], IO_DT, kind="ExternalInput").ap()
    b2 = nc.dram_tensor("b2", [P, MT], mybir.dt.float32, kind="ExternalInput").ap()
    out = nc.dram_tensor("out", [BPC, C, T], IO_DT, kind="ExternalOutput").ap()

    with ExitStack() as ctx:
        tc = ctx.enter_context(tile.TileContext(nc))
        wpool = ctx.enter_context(tc.tile_pool(name="w", bufs=1))
        bpool = ctx.enter_context(tc.tile_pool(name="bias", bufs=1))
        dpool = ctx.enter_context(tc.tile_pool(name="dummy", bufs=1))
        xpool = ctx.enter_context(tc.tile_pool(name="x", bufs=2))
        opool = ctx.enter_context(tc.tile_pool(name="o", bufs=4))
        pspool = ctx.enter_context(tc.tile_pool(name="ps", bufs=8, space="PSUM"))

        # HAM warmup: matmuls over scratch SBUF (contents irrelevant, result
        # never read) with no DMA dependencies — they run as soon as the PE
        # queue starts, while input DMAs are still in flight. The memset
        # satisfies the tile allocator's write-before-read requirement.
        dummy = dpool.tile([P, 256], IO_DT)
        nc.gpsimd.memset(dummy[:], 0)
        for i in range(NDUMMY):
            ps = pspool.tile([P, 256], mybir.dt.float32, tag="ps", name=f"psd_{i}")
            nc.tensor.matmul(ps[:], dummy[:, 0:P], dummy[:], start=True, stop=True)

        # All x chunks for both batches up front on the sync HWDGE queue:
        # one contiguous DMA per chunk covers all 4 K-tiles.
        xtiles = {}
        for b in range(BPC):
            off = 0
            for xi, xcols in enumerate(XCS):
                x = xpool.tile(
                    [P, KT, xcols], IO_DT, name=f"x_{b}_{xi}", tag=f"x{xi}", bufs=2
                )
                nc.sync.dma_start(
                    x[:], prex[b, :, bass.ds(KT * off, KT * xcols)]
                )
                xtiles[b, xi] = x
                off += xcols

        # Weights (one DMA per output-column tile, first one gates the first
        # matmul group) + bias on the scalar HWDGE queue, parallel with x.
        wtiles = []
        for mt in range(MT):
            w = wpool.tile([P, KT * P], IO_DT, name=f"w_{mt}")
            nc.scalar.dma_start(w[:], wt[mt])
            wtiles.append(w)
        btile = bpool.tile([P, MT], mybir.dt.float32)
        nc.scalar.dma_start(btile[:], b2[:])

        def wslice(kt, mt):
            return wtiles[mt][:, kt * P : (kt + 1) * P]

        for b in range(BPC):
            wi = 0
            obase = 0
            for og, nwin in enumerate(OG_SPLIT[b]):
                wins = WINDOWS[wi : wi + nwin]
                ocols = sum(w[2] for w in wins)
                otile = opool.tile([P, MT, ocols], IO_DT, name=f"o_{b}_{og}", tag="o")
                tail_og = b == BPC - 1 and og == len(OG_SPLIT[b]) - 1
                ooff = 0
                for xi, xoff, ncols in wins:
                    for mt in range(MT):
                        ps = pspool.tile([P, ncols], mybir.dt.float32, tag="ps")
                        for kt in range(KT):
                            nc.tensor.matmul(
                                ps[:],
                                wslice(kt, mt),
                                xtiles[b, xi][:, kt, xoff : xoff + ncols],
                                start=(kt == 0),
                                stop=(kt == KT - 1),
                            )
                        # W is pre-scaled by 2 on the host, so only + 2*bias
                        # remains; alternate DVE/ACT so neither engine binds.
                        dst = otile[:, mt, ooff : ooff + ncols]
                        bias_col = btile[:, mt : mt + 1]
                        if tail_og:
                            # Final group: evict in halves on both engines in
                            # parallel to shorten the serial tail.
                            h = ncols // 2
                            nc.vector.tensor_scalar_add(
                                dst[:, 0:h], ps[:, 0:h], bias_col
                            )
                            nc.scalar.activation(
                                dst[:, h:ncols],
                                ps[:, h:ncols],
                                mybir.ActivationFunctionType.Identity,
                                bias=bias_col,
                            )
                        elif mt % 2 == 0:
                            nc.vector.tensor_scalar_add(dst, ps[:], bias_col)
                        else:
                            nc.scalar.activation(
                                dst,
                                ps[:],
                                mybir.ActivationFunctionType.Identity,
                                bias=bias_col,
                            )
                        if tail_og:
                            # Per-M-tile store right after its eviction on the
                            # by-now-idle sync queue: the last HBM write is a
                            # small 128 KB transfer chasing the last matmul.
                            nc.sync.dma_start(
                                out[b, mt * P : (mt + 1) * P, bass.ds(obase, ocols)],
                                otile[:, mt, :],
                            )
                    ooff += ncols
                if not tail_og:
                    # One store per group covers all 4 M-tiles. Bulk stores
                    # ride the gpsimd SWDGE queue; the small second-to-last
                    # group uses scalar HWDGE for its lower fixed latency.
                    dst_d = out[b, :, bass.ds(obase, ocols)].rearrange(
                        "(mt p) j -> p mt j", mt=MT
                    )
                    small = b == BPC - 1 and og == len(OG_SPLIT[b]) - 2
                    eng = nc.scalar if small else nc.gpsimd
                    eng.dma_start(dst_d, otile[:])
                wi += nwin
                obase += ocols
    # The axon/PJRT exec path serializes nc as-is; finalize here so Bacc's
    # compile passes (register alloc, event-semaphore wait splitting) run.
    nc.finalize()
    return nc


def _blocked_x(pre16):
    """[B, C, T] -> [B, P, KT*T]: per chunk, [p, kt, j] flattened contiguously."""
    out = np.empty((B, P, KT * T), dtype=np.float16)
    off = 0
    for xcols in XCS:
        blk = pre16[:, :, off : off + xcols].reshape(B, KT, P, xcols)
        out[:, :, KT * off : KT * (off + xcols)] = (
            blk.transpose(0, 2, 1, 3).reshape(B, P, KT * xcols)
        )
        off += xcols
    return out


def kernel(pre, W_pre, b_pre):
    global LAST_RESULT
    pre16 = np.asarray(pre, dtype=np.float32).astype(np.float16)
    prex = _blocked_x(pre16)
    # Fold the reference's final y+y into the weights/bias: out = (2W)x + 2b.
    w2t = (np.asarray(W_pre, dtype=np.float32).T * 2.0).astype(np.float16)
    wt = np.ascontiguousarray(
        w2t.reshape(KT, P, MT, P).transpose(2, 1, 0, 3).reshape(MT, P, KT * P)
    )
    b2 = np.ascontiguousarray(
        (2.0 * np.asarray(b_pre, dtype=np.float32)).reshape(MT, P).T
    )
    if "nc" not in _cache:
        _cache["nc"] = _build()
    nc = _cache["nc"]
    in_maps = [
        {"prex": prex[i * BPC : (i + 1) * BPC], "wt": wt, "b2": b2}
        for i in range(NCORES)
    ]
    res = run_bass_kernel_spmd(nc, in_maps, list(range(NCORES)))
    LAST_RESULT = res
    return np.ascontiguousarray(
        np.concatenate([res.results[i]["out"] for i in range(NCORES)], axis=0)
    ).astype(np.float32)


# revision 7
# speedup vs baseline: 1.1091x; 1.1091x over previous
"""Trainium2 Bass kernel for y = 2*(einsum('bct,oc->bot', pre, W_pre) + b_pre).

Shapes (hardcoded): pre [16, 512, 4096] f32, W_pre [512, 512] f32, b_pre [512] f32.
Sharding: data-parallel over B across 8 cores (2 batches per core).

Per core: out[b, o, t] = 2*(sum_c W[o,c]*pre[b,c,t] + bias[o]) for 2 batches.
PE matmul computes lhsT.T @ rhs with lhsT = W.T tiles [K=128, M=128] and
rhs = pre column windows [K=128, N<=512]; 4 K-tiles accumulate into one
PSUM bank, then ScalarE/DVE apply out = psum + 2*bias on eviction
PSUM->SBUF (the reference's y+y is folded into W on the host).

I/O runs in fp16: per core f32 I/O moves 33.6 MB (~94 us at the 358 GB/s
HBM limit) while the PE needs only ~55 us at 1 cycle/row, so f32 is
DMA-bound; fp16 halves traffic and runs at the same PE rate.

Schedule notes (from NTFF traces):
- HWDGE dma_start costs ~650 ns fixed issue + descriptor-count-driven
  doorbell latency, so the host pre-blocks `pre` per column chunk
  ([p, kt, j] flattened) making every x chunk one contiguous-line DMA
  with few descriptors.
- SDMA engines round-robin *per packet* across queues, which starves
  small transfers racing big ones. Within one queue transfers are FIFO,
  so everything start-critical goes on the sync queue in consumption
  order: [W(mt0,mt1), x0, W(mt2,mt3), x1, x2, ...]. A single queue can
  still reach ~400 GB/s. Bias rides scalar, bulk stores gpsimd (SWDGE),
  tail stores sync/scalar.
- The PE HAM clock gate starts at 1.2 GHz and reaches 2.4 GHz only after
  ~3.4 us of sustained matmul activity. A burst of dummy matmuls on
  scratch SBUF issued before the main loop warms the PE during the
  ~2.8 us DMA head and bridges to first-data so the HAM busy window
  isn't reset by an idle gap.
- Tail: column chunks taper (…, 768, 256) so the final group is only 256
  cols; it evicts in 128-col halves on DVE+ACT in parallel and stores
  per M-tile (4 x 64 KB DMAs) on the by-then-idle sync queue, so the
  last HBM write chases the last matmul as closely as possible before
  the fixed ~2.5 us teardown barrier.
"""

import os
import sys

for _p in ("/opt/trn_rl_repo", "/root/.axon_site/_ro/trn_rl_repo"):
    if os.path.isdir(_p) and _p not in sys.path:
        sys.path.append(_p)

from contextlib import ExitStack

import numpy as np

import concourse.bass as bass
import concourse.tile as tile
from concourse import bacc, mybir
from concourse.bass_utils import run_bass_kernel_spmd

B, C, T = 16, 512, 4096  # batch, channels (in == out), sequence
NCORES = 8
BPC = B // NCORES  # batches per core
P = 128
KT = C // P  # contraction tiles
MT = C // P  # output-channel tiles
NCHUNK = 512  # max matmul moving-operand free dim (one PSUM bank of f32)
# Input DMA column chunks: tiny first chunks so the first matmul group's
# data lands early, big middle ones to amortize the fixed HWDGE issue
# cost, small final ones so the output tail is short.
XCS = [128, 128, 256, 512, 1024, 1024, 768, 256]
NDUMMY = 10  # warmup matmuls (N=256): ~10 * 236 ns cold bridges to first data

IO_DT = mybir.dt.float16  # matmul + DRAM I/O dtype

LAST_RESULT = None  # BassKernelResults of the most recent run (for test harness)
_cache = {}


def _windows():
    """Column windows (xi, xoff, ncols) covering [0, T) in matmul-N pieces."""
    ws = []
    for xi, xcols in enumerate(XCS):
        for o in range(0, xcols, NCHUNK):
            ws.append((xi, o, min(NCHUNK, xcols - o)))
    assert sum(w[2] for w in ws) == T
    return ws


# Output store groups as window-index ranges per batch. Batch 1 tapers so
# the final DMAs after the last matmul are small.
WINDOWS = _windows()  # 11 windows: 128,128,256,512x6,256,256
OG_SPLIT = {0: [6, 5], 1: [6, 2, 1, 1, 1]}  # cols 2048/2048, 2048/1024/512/256/256


def _build():
    # Bacc (not plain Bass): its finalize() runs move_matmul_waits_to_ldweights +
    # generate_event_semaphores, which walrus needs.
    nc = bacc.Bacc("TRN2", target_bir_lowering=False, debug=False, num_devices=NCORES)
    # Host layout: prex[b, p, 4*off + kt*xcols + j] for chunk at column off —
    # each chunk is a contiguous [128, 4*xcols] block (128 descriptors).
    prex = nc.dram_tensor("prex", [BPC, P, KT * T], IO_DT, kind="ExternalInput").ap()
    # Host layout: wt[mt, p, kt*128 + m] = 2*W.T[kt*128+p, mt*128+m].
    wt = nc.dram_tensor("wt", [MT, P, KT * P], IO_DT, kind="ExternalInput").ap()
    b2 = nc.dram_tensor("b2", [P, MT], mybir.dt.float32, kind="ExternalInput").ap()
    out = nc.dram_tensor("out", [BPC, C, T], IO_DT, kind="ExternalOutput").ap()

    with ExitStack() as ctx:
        tc = ctx.enter_context(tile.TileContext(nc))
        wpool = ctx.enter_context(tc.tile_pool(name="w", bufs=1))
        bpool = ctx.enter_context(tc.tile_pool(name="bias", bufs=1))
        dpool = ctx.enter_context(tc.tile_pool(name="dummy", bufs=1))
        xpool = ctx.enter_context(tc.tile_pool(name="x", bufs=2))
        opool = ctx.enter_context(tc.tile_pool(name="o", bufs=4))
        pspool = ctx.enter_context(tc.tile_pool(name="ps", bufs=8, space="PSUM"))

        # HAM warmup: matmuls over scratch SBUF (contents irrelevant, result
        # never read) with no DMA dependencies — they run as soon as the PE
        # queue starts, while input DMAs are still in flight. The memset
        # satisfies the tile allocator's write-before-read requirement.
        dummy = dpool.tile([P, 256], IO_DT)
        nc.gpsimd.memset(dummy[:], 0)
        for i in range(NDUMMY):
            ps = pspool.tile([P, 256], mybir.dt.float32, tag="ps", name=f"psd_{i}")
            nc.tensor.matmul(ps[:], dummy[:, 0:P], dummy[:], start=True, stop=True)

        # Start-critical loads in exact consumption order on the sync queue
        # (FIFO within a queue = guaranteed transfer priority): weights for
        # mt0/mt1, then x chunk 0, then weights mt2/mt3, then the rest of x.
        wab = []
        for h in range(2):
            w = wpool.tile([P, 2, KT * P], IO_DT, name=f"w_{h}")
            wab.append(w)
        xtiles = {}

        def load_x(b, xi, off):
            x = xpool.tile(
                [P, KT, XCS[xi]], IO_DT, name=f"x_{b}_{xi}", tag=f"x{xi}", bufs=2
            )
            nc.sync.dma_start(x[:], prex[b, :, bass.ds(KT * off, KT * XCS[xi])])
            xtiles[b, xi] = x

        xoffs = np.cumsum([0] + XCS).tolist()
        nc.sync.dma_start(wab[0][:], wt[0:2].rearrange("m p c -> p m c"))
        load_x(0, 0, xoffs[0])
        nc.sync.dma_start(wab[1][:], wt[2:4].rearrange("m p c -> p m c"))
        for xi in range(1, len(XCS)):
            load_x(0, xi, xoffs[xi])
        for xi in range(len(XCS)):
            load_x(1, xi, xoffs[xi])

        # Bias on the scalar HWDGE queue (tiny, uncontended).
        btile = bpool.tile([P, MT], mybir.dt.float32)
        nc.scalar.dma_start(btile[:], b2[:])

        def wslice(kt, mt):
            return wab[mt // 2][:, mt % 2, kt * P : (kt + 1) * P]

        for b in range(BPC):
            wi = 0
            obase = 0
            for og, nwin in enumerate(OG_SPLIT[b]):
                wins = WINDOWS[wi : wi + nwin]
                ocols = sum(w[2] for w in wins)
                otile = opool.tile([P, MT, ocols], IO_DT, name=f"o_{b}_{og}", tag="o")
                tail_og = b == BPC - 1 and og == len(OG_SPLIT[b]) - 1
                ooff = 0
                for xi, xoff, ncols in wins:
                    for mt in range(MT):
                        ps = pspool.tile([P, ncols], mybir.dt.float32, tag="ps")
                        for kt in range(KT):
                            nc.tensor.matmul(
                                ps[:],
                                wslice(kt, mt),
                                xtiles[b, xi][:, kt, xoff : xoff + ncols],
                                start=(kt == 0),
                                stop=(kt == KT - 1),
                            )
                        # W is pre-scaled by 2 on the host, so only + 2*bias
                        # remains; alternate DVE/ACT so neither engine binds.
                        dst = otile[:, mt, ooff : ooff + ncols]
                        bias_col = btile[:, mt : mt + 1]
                        if tail_og:
                            # Final group: evict in halves on both engines in
                            # parallel to shorten the serial tail.
                            h = ncols // 2
                            nc.vector.tensor_scalar_add(
                                dst[:, 0:h], ps[:, 0:h], bias_col
                            )
                            nc.scalar.activation(
                                dst[:, h:ncols],
                                ps[:, h:ncols],
                                mybir.ActivationFunctionType.Identity,
                                bias=bias_col,
                            )
                        elif mt % 2 == 0:
                            nc.vector.tensor_scalar_add(dst, ps[:], bias_col)
                        else:
                            nc.scalar.activation(
                                dst,
                                ps[:],
                                mybir.ActivationFunctionType.Identity,
                                bias=bias_col,
                            )
                        if tail_og:
                            # Per-M-tile store right after its eviction on the
                            # by-now-idle sync queue: the last HBM write is a
                            # small 64 KB transfer chasing the last matmul.
                            nc.sync.dma_start(
                                out[b, mt * P : (mt + 1) * P, bass.ds(obase, ocols)],
                                otile[:, mt, :],
                            )
                    ooff += ncols
                if not tail_og:
                    # One store per group covers all 4 M-tiles. Bulk stores
                    # ride the gpsimd SWDGE queue; the small late groups use
                    # scalar HWDGE for its lower fixed latency.
                    dst_d = out[b, :, bass.ds(obase, ocols)].rearrange(
                        "(mt p) j -> p mt j", mt=MT
                    )
                    small = b == BPC - 1 and og >= len(OG_SPLIT[b]) - 3
                    eng = nc.scalar if small else nc.gpsimd
                    eng.dma_start(dst_d, otile[:])
                wi += nwin
                obase += ocols
    # The axon/PJRT exec path serializes nc as-is; finalize here so Bacc's
    # compile passes (register alloc, event-semaphore wait splitting) run.
    nc.finalize()
    return nc


def _blocked_x(pre16):
    """[B, C, T] -> [B, P, KT*T]: per chunk, [p, kt, j] flattened contiguously."""
    out = np.empty((B, P, KT * T), dtype=np.float16)
    off = 0
    for xcols in XCS:
        blk = pre16[:, :, off : off + xcols].reshape(B, KT, P, xcols)
        out[:, :, KT * off : KT * (off + xcols)] = (
            blk.transpose(0, 2, 1, 3).reshape(B, P, KT * xcols)
        )
        off += xcols
    return out


def kernel(pre, W_pre, b_pre):
    global LAST_RESULT
    pre16 = np.asarray(pre, dtype=np.float32).astype(np.float16)
    prex = _blocked_x(pre16)
    # Fold the reference's final y+y into the weights/bias: out = (2W)x + 2b.
    w2t = (np.asarray(W_pre, dtype=np.float32).T * 2.0).astype(np.float16)
    wt = np.ascontiguousarray(
        w2t.reshape(KT, P, MT, P).transpose(2, 1, 0, 3).reshape(MT, P, KT * P)
    )
    b2 = np.ascontiguousarray(
        (2.0 * np.asarray(b_pre, dtype=np.float32)).reshape(MT, P).T
    )
    if "nc" not in _cache:
        _cache["nc"] = _build()
    nc = _cache["nc"]
    in_maps = [
        {"prex": prex[i * BPC : (i + 1) * BPC], "wt": wt, "b2": b2}
        for i in range(NCORES)
    ]
    res = run_bass_kernel_spmd(nc, in_maps, list(range(NCORES)))
    LAST_RESULT = res
    return np.ascontiguousarray(
        np.concatenate([res.results[i]["out"] for i in range(NCORES)], axis=0)
    ).astype(np.float32)


# revision 14
# speedup vs baseline: 1.1268x; 1.0160x over previous
"""Trainium2 Bass kernel for y = 2*(einsum('bct,oc->bot', pre, W_pre) + b_pre).

Shapes (hardcoded): pre [16, 512, 4096] f32, W_pre [512, 512] f32, b_pre [512] f32.
Sharding: data-parallel over B across 8 cores (2 batches per core).

Per core: out[b, o, t] = 2*(sum_c W[o,c]*pre[b,c,t] + bias[o]) for 2 batches.
PE matmul computes lhsT.T @ rhs with lhsT = W.T tiles [K=128, M=128] and
rhs = pre column windows [K=128, N<=512]; 4 K-tiles accumulate into one
PSUM bank, then ScalarE/DVE apply out = psum + 2*bias on eviction
PSUM->SBUF (the reference's y+y is folded into W on the host).

I/O runs in fp16: per core f32 I/O moves 33.6 MB (~94 us at the 358 GB/s
HBM limit) while the PE needs only ~55 us at 1 cycle/row, so f32 is
DMA-bound; fp16 halves traffic and runs at the same PE rate.

Schedule notes (from NTFF traces):
- HWDGE dma_start costs ~650 ns fixed issue + descriptor-count-driven
  doorbell latency, so the host pre-blocks `pre` per column chunk
  ([p, kt, j] flattened) making every x chunk one contiguous-line DMA
  with few descriptors.
- SDMA engines round-robin *per packet* across queues, which starves
  small transfers racing big ones. Within one queue transfers are FIFO,
  so everything start-critical goes on the sync queue in consumption
  order: [W(mt0,mt1), x0, W(mt2,mt3), x1, x2, ...]. A single queue can
  still reach ~400 GB/s. Bias rides scalar, bulk stores gpsimd (SWDGE),
  tail stores sync/scalar.
- The PE HAM clock gate starts at 1.2 GHz and reaches 2.4 GHz only after
  ~3.4 us of sustained matmul activity. A burst of dummy matmuls on
  scratch SBUF issued before the main loop warms the PE during the
  ~2.8 us DMA head and bridges to first-data so the HAM busy window
  isn't reset by an idle gap.
- Tail: column chunks taper (…, 768, 256) so the final group is only 256
  cols; it evicts in 128-col halves on DVE+ACT in parallel and stores
  per M-tile (4 x 64 KB DMAs) on the by-then-idle sync queue, so the
  last HBM write chases the last matmul as closely as possible before
  the fixed ~2.5 us teardown barrier.
"""

import os
import sys

for _p in ("/opt/trn_rl_repo", "/root/.axon_site/_ro/trn_rl_repo"):
    if os.path.isdir(_p) and _p not in sys.path:
        sys.path.append(_p)

from contextlib import ExitStack

import numpy as np

import concourse.bass as bass
import concourse.tile as tile
from concourse import bacc, mybir
from concourse.bass_utils import run_bass_kernel_spmd

B, C, T = 16, 512, 4096  # batch, channels (in == out), sequence
NCORES = 8
BPC = B // NCORES  # batches per core
P = 128
KT = C // P  # contraction tiles
MT = C // P  # output-channel tiles
NCHUNK = 512  # max matmul moving-operand free dim (one PSUM bank of f32)
# Input DMA column chunks: tiny first chunks so the first matmul group's
# data lands early, big middle ones to amortize the fixed HWDGE issue
# cost, small final ones so the output tail is short.
XCS = [128, 128, 256, 512, 1024, 1024, 768, 256]
NDUMMY = 9  # warmup matmuls (N=256): ~9 * 236 ns cold bridges to first data

IO_DT = mybir.dt.float16  # matmul + DRAM I/O dtype

LAST_RESULT = None  # BassKernelResults of the most recent run (for test harness)
_cache = {}


def _windows():
    """Column windows (xi, xoff, ncols) covering [0, T) in matmul-N pieces."""
    ws = []
    for xi, xcols in enumerate(XCS):
        for o in range(0, xcols, NCHUNK):
            ws.append((xi, o, min(NCHUNK, xcols - o)))
    assert sum(w[2] for w in ws) == T
    return ws


# Output store groups as window-index ranges per batch. Batch 1 tapers so
# the final DMAs after the last matmul are small.
WINDOWS = _windows()  # 11 windows: 128,128,256,512x6,256,256
OG_SPLIT = {0: [6, 5], 1: [6, 2, 1, 1, 1]}  # cols 2048/2048, 2048/1024/512/256/256


def _build():
    # Bacc (not plain Bass): its finalize() runs move_matmul_waits_to_ldweights +
    # generate_event_semaphores, which walrus needs.
    nc = bacc.Bacc("TRN2", target_bir_lowering=False, debug=False, num_devices=NCORES)
    # Host layout: prex[b, p, 4*off + kt*xcols + j] for chunk at column off —
    # each chunk is a contiguous [128, 4*xcols] block (128 descriptors).
    prex = nc.dram_tensor("prex", [BPC, P, KT * T], IO_DT, kind="ExternalInput").ap()
    # Host layout: wt[h, p, mh*512 + kt*128 + m] = 2*W.T[kt*128+p, (2h+mh)*128+m]
    # — each half (output columns mt=2h..2h+1) is one contiguous [128, 4 KB]
    # block loading in a single 128-descriptor DMA.
    wt = nc.dram_tensor("wt", [2, P, 2 * KT * P], IO_DT, kind="ExternalInput").ap()
    b2 = nc.dram_tensor("b2", [P, MT], mybir.dt.float32, kind="ExternalInput").ap()
    out = nc.dram_tensor("out", [BPC, C, T], IO_DT, kind="ExternalOutput").ap()

    with ExitStack() as ctx:
        tc = ctx.enter_context(tile.TileContext(nc))
        wpool = ctx.enter_context(tc.tile_pool(name="w", bufs=1))
        bpool = ctx.enter_context(tc.tile_pool(name="bias", bufs=1))
        dpool = ctx.enter_context(tc.tile_pool(name="dummy", bufs=1))
        xpool = ctx.enter_context(tc.tile_pool(name="x", bufs=2))
        opool = ctx.enter_context(tc.tile_pool(name="o", bufs=4))
        pspool = ctx.enter_context(tc.tile_pool(name="ps", bufs=8, space="PSUM"))

        # HAM warmup: matmuls over scratch SBUF (contents irrelevant, result
        # never read) with no DMA dependencies — they run as soon as the PE
        # queue starts, while input DMAs are still in flight. The memset
        # satisfies the tile allocator's write-before-read requirement.
        dummy = dpool.tile([P, 256], IO_DT)
        nc.gpsimd.memset(dummy[:], 0)
        for i in range(NDUMMY):
            ps = pspool.tile([P, 256], mybir.dt.float32, tag="ps", name=f"psd_{i}")
            nc.tensor.matmul(ps[:], dummy[:, 0:P], dummy[:], start=True, stop=True)

        # x chunks stream on the sync queue in consumption order (FIFO within
        # a queue = guaranteed priority); weights go on the scalar queue in
        # two fat-line halves (mt0/1 then mt2/3) so they transfer in parallel
        # with x chunk 0 and the first matmul group can start ~9.5 us in.
        wab = []
        for h in range(2):
            w = wpool.tile([P, 2 * KT * P], IO_DT, name=f"w_{h}")
            nc.scalar.dma_start(w[:], wt[h])
            wab.append(w)
        btile = bpool.tile([P, MT], mybir.dt.float32)
        nc.scalar.dma_start(btile[:], b2[:])

        xtiles = {}

        def load_x(b, xi, off):
            x = xpool.tile(
                [P, KT, XCS[xi]], IO_DT, name=f"x_{b}_{xi}", tag=f"x{xi}", bufs=2
            )
            nc.sync.dma_start(x[:], prex[b, :, bass.ds(KT * off, KT * XCS[xi])])
            xtiles[b, xi] = x

        xoffs = np.cumsum([0] + XCS).tolist()
        for b in range(BPC):
            for xi in range(len(XCS)):
                load_x(b, xi, xoffs[xi])

        def wslice(kt, mt):
            base = (mt % 2) * KT * P + kt * P
            return wab[mt // 2][:, base : base + P]

        for b in range(BPC):
            wi = 0
            obase = 0
            for og, nwin in enumerate(OG_SPLIT[b]):
                wins = WINDOWS[wi : wi + nwin]
                ocols = sum(w[2] for w in wins)
                otile = opool.tile([P, MT, ocols], IO_DT, name=f"o_{b}_{og}", tag="o")
                tail_og = b == BPC - 1 and og == len(OG_SPLIT[b]) - 1
                ooff = 0
                for xi, xoff, ncols in wins:
                    for mt in range(MT):
                        ps = pspool.tile([P, ncols], mybir.dt.float32, tag="ps")
                        for kt in range(KT):
                            nc.tensor.matmul(
                                ps[:],
                                wslice(kt, mt),
                                xtiles[b, xi][:, kt, xoff : xoff + ncols],
                                start=(kt == 0),
                                stop=(kt == KT - 1),
                            )
                        # W is pre-scaled by 2 on the host, so only + 2*bias
                        # remains; alternate DVE/ACT so neither engine binds.
                        dst = otile[:, mt, ooff : ooff + ncols]
                        bias_col = btile[:, mt : mt + 1]
                        if mt % 2 == 0:
                            nc.vector.tensor_scalar_add(dst, ps[:], bias_col)
                        else:
                            nc.scalar.activation(
                                dst,
                                ps[:],
                                mybir.ActivationFunctionType.Identity,
                                bias=bias_col,
                            )
                        if tail_og:
                            # Per-M-tile store right after its eviction; the
                            # even (DVE-evicted) tiles store from the idle
                            # sync queue, the odd (ACT-evicted) ones from
                            # scalar, so the two HWDGE generators overlap and
                            # the last HBM write is a 64 KB transfer chasing
                            # the last matmul.
                            eng = nc.sync if mt % 2 == 0 else nc.scalar
                            eng.dma_start(
                                out[b, mt * P : (mt + 1) * P, bass.ds(obase, ocols)],
                                otile[:, mt, :],
                            )
                    ooff += ncols
                if not tail_og:
                    # One store per group covers all 4 M-tiles. Bulk stores
                    # ride the gpsimd SWDGE queue; the small late groups use
                    # scalar HWDGE for its lower fixed latency.
                    dst_d = out[b, :, bass.ds(obase, ocols)].rearrange(
                        "(mt p) j -> p mt j", mt=MT
                    )
                    small = b == BPC - 1 and og >= len(OG_SPLIT[b]) - 3
                    eng = nc.scalar if small else nc.gpsimd
                    eng.dma_start(dst_d, otile[:])
                wi += nwin
                obase += ocols
    # The axon/PJRT exec path serializes nc as-is; finalize here so Bacc's
    # compile passes (register alloc, event-semaphore wait splitting) run.
    nc.finalize()
    return nc


def _blocked_x(pre16):
    """[B, C, T] -> [B, P, KT*T]: per chunk, [p, kt, j] flattened contiguously."""
    out = np.empty((B, P, KT * T), dtype=np.float16)
    off = 0
    for xcols in XCS:
        blk = pre16[:, :, off : off + xcols].reshape(B, KT, P, xcols)
        out[:, :, KT * off : KT * (off + xcols)] = (
            blk.transpose(0, 2, 1, 3).reshape(B, P, KT * xcols)
        )
        off += xcols
    return out


def kernel(pre, W_pre, b_pre):
    global LAST_RESULT
    pre16 = np.asarray(pre, dtype=np.float32).astype(np.float16)
    prex = _blocked_x(pre16)
    # Fold the reference's final y+y into the weights/bias: out = (2W)x + 2b.
    w2t = (np.asarray(W_pre, dtype=np.float32).T * 2.0).astype(np.float16)
    wt = np.ascontiguousarray(
        w2t.reshape(KT, P, MT, P)
        .transpose(2, 1, 0, 3)  # [mt, p, kt, m]
        .reshape(2, 2, P, KT, P)
        .transpose(0, 2, 1, 3, 4)  # [h, p, mh, kt, m]
        .reshape(2, P, 2 * KT * P)
    )
    b2 = np.ascontiguousarray(
        (2.0 * np.asarray(b_pre, dtype=np.float32)).reshape(MT, P).T
    )
    if "nc" not in _cache:
        _cache["nc"] = _build()
    nc = _cache["nc"]
    in_maps = [
        {"prex": prex[i * BPC : (i + 1) * BPC], "wt": wt, "b2": b2}
        for i in range(NCORES)
    ]
    res = run_bass_kernel_spmd(nc, in_maps, list(range(NCORES)))
    LAST_RESULT = res
    return np.ascontiguousarray(
        np.concatenate([res.results[i]["out"] for i in range(NCORES)], axis=0)
    ).astype(np.float32)


# revision 17
# speedup vs baseline: 1.1329x; 1.0054x over previous
"""Trainium2 Bass kernel for y = 2*(einsum('bct,oc->bot', pre, W_pre) + b_pre).

Shapes (hardcoded): pre [16, 512, 4096] f32, W_pre [512, 512] f32, b_pre [512] f32.
Sharding: data-parallel over B across 8 cores (2 batches per core).

Per core: out[b, o, t] = 2*(sum_c W[o,c]*pre[b,c,t] + bias[o]) for 2 batches.
PE matmul computes lhsT.T @ rhs with lhsT = W.T tiles [K=128, M=128] and
rhs = pre column windows [K=128, N<=512]; 4 K-tiles accumulate into one
PSUM bank, then ScalarE/DVE apply out = psum + 2*bias on eviction
PSUM->SBUF (the reference's y+y is folded into W on the host).

I/O runs in fp16: per core f32 I/O moves 33.6 MB (~94 us at the 358 GB/s
HBM limit) while the PE needs only ~55 us at 1 cycle/row, so f32 is
DMA-bound; fp16 halves traffic and runs at the same PE rate.

Schedule notes (from NTFF traces):
- HWDGE dma_start costs ~650 ns fixed issue + descriptor-count-driven
  doorbell latency, so the host pre-blocks `pre` per column chunk
  ([p, kt, j] flattened) making every x chunk one contiguous-line DMA
  with few descriptors.
- SDMA engines round-robin *per packet* across queues, which starves
  small transfers racing big ones. Within one queue transfers are FIFO,
  so everything start-critical goes on the sync queue in consumption
  order: [W(mt0,mt1), x0, W(mt2,mt3), x1, x2, ...]. A single queue can
  still reach ~400 GB/s. Bias rides scalar, bulk stores gpsimd (SWDGE),
  tail stores sync/scalar.
- The PE HAM clock gate starts at 1.2 GHz and reaches 2.4 GHz only after
  ~3.4 us of sustained matmul activity. A burst of dummy matmuls on
  scratch SBUF issued before the main loop warms the PE during the
  ~2.8 us DMA head and bridges to first-data so the HAM busy window
  isn't reset by an idle gap.
- Tail: column chunks taper (…, 768, 256) so the final group is only 256
  cols; it evicts in 128-col halves on DVE+ACT in parallel and stores
  per M-tile (4 x 64 KB DMAs) on the by-then-idle sync queue, so the
  last HBM write chases the last matmul as closely as possible before
  the fixed ~2.5 us teardown barrier.
"""

import os
import sys

for _p in ("/opt/trn_rl_repo", "/root/.axon_site/_ro/trn_rl_repo"):
    if os.path.isdir(_p) and _p not in sys.path:
        sys.path.append(_p)

from contextlib import ExitStack

import numpy as np

import concourse.bass as bass
import concourse.tile as tile
from concourse import bacc, mybir
from concourse.bass_utils import run_bass_kernel_spmd

B, C, T = 16, 512, 4096  # batch, channels (in == out), sequence
NCORES = 8
BPC = B // NCORES  # batches per core
P = 128
KT = C // P  # contraction tiles
MT = C // P  # output-channel tiles
NCHUNK = 512  # max matmul moving-operand free dim (one PSUM bank of f32)
# Input DMA column chunks: tiny first chunks so the first matmul group's
# data lands early, big middle ones to amortize the fixed HWDGE issue
# cost, small final ones so the output tail is short.
XCS = [128, 128, 256, 512, 1024, 1024, 768, 256]
NDUMMY = 7  # warmup matmuls (N=512): ~7 * 427 ns cold bridges to first data

IO_DT = mybir.dt.float16  # matmul + DRAM I/O dtype

LAST_RESULT = None  # BassKernelResults of the most recent run (for test harness)
_cache = {}


def _windows():
    """Column windows (xi, xoff, ncols) covering [0, T) in matmul-N pieces."""
    ws = []
    for xi, xcols in enumerate(XCS):
        for o in range(0, xcols, NCHUNK):
            ws.append((xi, o, min(NCHUNK, xcols - o)))
    assert sum(w[2] for w in ws) == T
    return ws


# Output store groups as window-index ranges per batch. Batch 1 tapers so
# the final DMAs after the last matmul are small.
WINDOWS = _windows()  # 11 windows: 128,128,256,512x6,256,256
OG_SPLIT = {0: [6, 5], 1: [6, 2, 1, 1, 1]}  # cols 2048/2048, 2048/1024/512/256/256


def _build():
    # Bacc (not plain Bass): its finalize() runs move_matmul_waits_to_ldweights +
    # generate_event_semaphores, which walrus needs.
    nc = bacc.Bacc("TRN2", target_bir_lowering=False, debug=False, num_devices=NCORES)
    # Host layout: prex[b, p, 4*off + kt*xcols + j] for chunk at column off —
    # each chunk is a contiguous [128, 4*xcols] block (128 descriptors).
    prex = nc.dram_tensor("prex", [BPC, P, KT * T], IO_DT, kind="ExternalInput").ap()
    # Host layout: wt[h, p, mh*512 + kt*128 + m] = 2*W.T[kt*128+p, (2h+mh)*128+m]
    # — each half (output columns mt=2h..2h+1) is one contiguous [128, 4 KB]
    # block loading in a single 128-descriptor DMA.
    wt = nc.dram_tensor("wt", [2, P, 2 * KT * P], IO_DT, kind="ExternalInput").ap()
    b2 = nc.dram_tensor("b2", [P, MT], mybir.dt.float32, kind="ExternalInput").ap()
    out = nc.dram_tensor("out", [BPC, C, T], IO_DT, kind="ExternalOutput").ap()

    with ExitStack() as ctx:
        tc = ctx.enter_context(tile.TileContext(nc))
        wpool = ctx.enter_context(tc.tile_pool(name="w", bufs=1))
        bpool = ctx.enter_context(tc.tile_pool(name="bias", bufs=1))
        dpool = ctx.enter_context(tc.tile_pool(name="dummy", bufs=1))
        xpool = ctx.enter_context(tc.tile_pool(name="x", bufs=2))
        opool = ctx.enter_context(tc.tile_pool(name="o", bufs=4))
        pspool = ctx.enter_context(tc.tile_pool(name="ps", bufs=8, space="PSUM"))

        # HAM warmup: matmuls over scratch SBUF (contents irrelevant, result
        # never read) with no DMA dependencies — they run as soon as the PE
        # queue starts, while input DMAs are still in flight. The memset
        # satisfies the tile allocator's write-before-read requirement.
        dummy = dpool.tile([P, NCHUNK], IO_DT)
        nc.gpsimd.memset(dummy[:], 0)
        for i in range(NDUMMY):
            ps = pspool.tile([P, NCHUNK], mybir.dt.float32, tag="ps", name=f"psd_{i}")
            nc.tensor.matmul(ps[:], dummy[:, 0:P], dummy[:], start=True, stop=True)

        # x chunks stream on the sync queue in consumption order (FIFO within
        # a queue = guaranteed priority); weights go on the scalar queue in
        # two fat-line halves (mt0/1 then mt2/3) so they transfer in parallel
        # with x chunk 0 and the first matmul group can start ~9.5 us in.
        wab = []
        for h in range(2):
            w = wpool.tile([P, 2 * KT * P], IO_DT, name=f"w_{h}")
            nc.scalar.dma_start(w[:], wt[h])
            wab.append(w)
        btile = bpool.tile([P, MT], mybir.dt.float32)
        nc.scalar.dma_start(btile[:], b2[:])

        xtiles = {}

        def load_x(b, xi, off):
            x = xpool.tile(
                [P, KT, XCS[xi]], IO_DT, name=f"x_{b}_{xi}", tag=f"x{xi}", bufs=2
            )
            nc.sync.dma_start(x[:], prex[b, :, bass.ds(KT * off, KT * XCS[xi])])
            xtiles[b, xi] = x

        xoffs = np.cumsum([0] + XCS).tolist()
        for b in range(BPC):
            for xi in range(len(XCS)):
                load_x(b, xi, xoffs[xi])

        def wslice(kt, mt):
            base = (mt % 2) * KT * P + kt * P
            return wab[mt // 2][:, base : base + P]

        for b in range(BPC):
            wi = 0
            obase = 0
            for og, nwin in enumerate(OG_SPLIT[b]):
                wins = WINDOWS[wi : wi + nwin]
                ocols = sum(w[2] for w in wins)
                otile = opool.tile([P, MT, ocols], IO_DT, name=f"o_{b}_{og}", tag="o")
                tail_og = b == BPC - 1 and og == len(OG_SPLIT[b]) - 1
                ooff = 0
                for xi, xoff, ncols in wins:
                    for mt in range(MT):
                        ps = pspool.tile([P, ncols], mybir.dt.float32, tag="ps")
                        for kt in range(KT):
                            nc.tensor.matmul(
                                ps[:],
                                wslice(kt, mt),
                                xtiles[b, xi][:, kt, xoff : xoff + ncols],
                                start=(kt == 0),
                                stop=(kt == KT - 1),
                            )
                        # W is pre-scaled by 2 on the host, so only + 2*bias
                        # remains; alternate DVE/ACT so neither engine binds.
                        dst = otile[:, mt, ooff : ooff + ncols]
                        bias_col = btile[:, mt : mt + 1]
                        if mt % 2 == 0:
                            nc.vector.tensor_scalar_add(dst, ps[:], bias_col)
                        else:
                            nc.scalar.activation(
                                dst,
                                ps[:],
                                mybir.ActivationFunctionType.Identity,
                                bias=bias_col,
                            )
                    ooff += ncols
                if tail_og:
                    # Per-M-tile stores issued after all evictions (so no
                    # store's HWDGE generation delays a later eviction on the
                    # same queue); even (DVE-evicted) tiles store from the
                    # idle sync queue, odd (ACT-evicted) ones from scalar, so
                    # the two HWDGE generators overlap and the last HBM write
                    # is a 64 KB transfer chasing the last matmul.
                    for mt in range(MT):
                        eng = nc.sync if mt % 2 == 0 else nc.scalar
                        eng.dma_start(
                            out[b, mt * P : (mt + 1) * P, bass.ds(obase, ocols)],
                            otile[:, mt, :],
                        )
                else:
                    # One store per group covers all 4 M-tiles. Bulk stores
                    # ride the gpsimd SWDGE queue; the small late groups use
                    # scalar HWDGE for its lower fixed latency.
                    dst_d = out[b, :, bass.ds(obase, ocols)].rearrange(
                        "(mt p) j -> p mt j", mt=MT
                    )
                    small = b == BPC - 1 and og >= len(OG_SPLIT[b]) - 3
                    eng = nc.scalar if small else nc.gpsimd
                    eng.dma_start(dst_d, otile[:])
                wi += nwin
                obase += ocols
    # The axon/PJRT exec path serializes nc as-is; finalize here so Bacc's
    # compile passes (register alloc, event-semaphore wait splitting) run.
    nc.finalize()
    return nc


def _blocked_x(pre16):
    """[B, C, T] -> [B, P, KT*T]: per chunk, [p, kt, j] flattened contiguously."""
    out = np.empty((B, P, KT * T), dtype=np.float16)
    off = 0
    for xcols in XCS:
        blk = pre16[:, :, off : off + xcols].reshape(B, KT, P, xcols)
        out[:, :, KT * off : KT * (off + xcols)] = (
            blk.transpose(0, 2, 1, 3).reshape(B, P, KT * xcols)
        )
        off += xcols
    return out


def kernel(pre, W_pre, b_pre):
    global LAST_RESULT
    pre16 = np.asarray(pre, dtype=np.float32).astype(np.float16)
    prex = _blocked_x(pre16)
    # Fold the reference's final y+y into the weights/bias: out = (2W)x + 2b.
    w2t = (np.asarray(W_pre, dtype=np.float32).T * 2.0).astype(np.float16)
    wt = np.ascontiguousarray(
        w2t.reshape(KT, P, MT, P)
        .transpose(2, 1, 0, 3)  # [mt, p, kt, m]
        .reshape(2, 2, P, KT, P)
        .transpose(0, 2, 1, 3, 4)  # [h, p, mh, kt, m]
        .reshape(2, P, 2 * KT * P)
    )
    b2 = np.ascontiguousarray(
        (2.0 * np.asarray(b_pre, dtype=np.float32)).reshape(MT, P).T
    )
    if "nc" not in _cache:
        _cache["nc"] = _build()
    nc = _cache["nc"]
    in_maps = [
        {"prex": prex[i * BPC : (i + 1) * BPC], "wt": wt, "b2": b2}
        for i in range(NCORES)
    ]
    res = run_bass_kernel_spmd(nc, in_maps, list(range(NCORES)))
    LAST_RESULT = res
    return np.ascontiguousarray(
        np.concatenate([res.results[i]["out"] for i in range(NCORES)], axis=0)
    ).astype(np.float32)
